# revision 4
# baseline (speedup 1.0000x reference)
"""Trainium2 Bass kernel for DenseRNN.

Computation (reference):
    h_{t} = tanh([x_t, h_{t-1}] @ W_fc.T + b_fc)   for t in 0..T-1
    y = h_T @ W_out.T + b_out
    returns (y, h_T)

Strategy (8-way data-parallel over batch, 32 batches/core):
  - Split W_fc into W_x (x part) and W_h (recurrent part), both bf16.
  - The x contribution ("xproj") is computed in windows of 16 timesteps with
    full-efficiency matmuls (N=512 moving), interleaved into the scan stream.
  - The recurrent scan keeps h in a "folded" transposed layout
    F[p, j*32+b] = h[b, j*128+p] so each step is 64 [128,128]x[128,32]
    matmuls (weights stationary, h moving) with no transposes anywhere.
  - tanh on the scalar engine reading PSUM directly after a DVE add of the
    precomputed xproj (+bias folded in at xproj eviction).
  - Epilogue: y matmul off the folded h (bias via a K=1 matmul with ones),
    h unfolded with 8 PE transposes.

All matmul inputs bf16 (fp32 accumulate); measured end-to-end rel err ~4e-3.
"""

import sys

sys.path.insert(0, "/opt/trn_rl_repo")

from contextlib import ExitStack

import ml_dtypes
import numpy as np

import concourse.bass as bass  # noqa: F401
import concourse.mybir as mybir
import concourse.tile as tile
from concourse import bacc
from concourse.bass import ds, ts
from concourse.masks import make_identity

BF16 = mybir.dt.bfloat16
F32 = mybir.dt.float32
TANH = mybir.ActivationFunctionType.Tanh
IDENT = mybir.ActivationFunctionType.Identity

B_FULL, T_FULL, D_IN, D_H, D_OUT = 256, 512, 512, 1024, 512
NCORES = 8
BSH = B_FULL // NCORES  # 32 batches per core
WIN = 16  # timesteps per xproj window
KC = 8  # k chunks of 128 over D_H (recurrent contraction)
JC = 8  # output chunks of 128 over D_H
XK = 4  # k chunks of 128 over D_IN


def build_nc(T=T_FULL, debug=False):
    NW = T // WIN
    NB = NW // 2  # For_i bodies, 2 windows each
    NWP = NW + 3  # padded windows in x_pre (lookahead DMA reads)

    nc = bacc.Bacc("TRN2", target_bir_lowering=False, debug=debug)
    xp = nc.dram_tensor("xp", [NWP * XK * 128, WIN * BSH], BF16, kind="ExternalInput").ap()
    hf = nc.dram_tensor("hf", [128, KC * BSH], BF16, kind="ExternalInput").ap()
    wfc = nc.dram_tensor("wfc", [D_IN + D_H, D_H], BF16, kind="ExternalInput").ap()
    bfc = nc.dram_tensor("bfc", [128, JC], F32, kind="ExternalInput").ap()
    wout = nc.dram_tensor("wout", [D_H, D_OUT], BF16, kind="ExternalInput").ap()
    bout = nc.dram_tensor("bout", [1, D_OUT], F32, kind="ExternalInput").ap()
    y = nc.dram_tensor("y", [BSH, D_OUT], F32, kind="ExternalOutput").ap()
    hout = nc.dram_tensor("hout", [BSH, D_H], F32, kind="ExternalOutput").ap()

    with tile.TileContext(nc) as tc, ExitStack() as ctx:
        wp = ctx.enter_context(tc.tile_pool(name="w", bufs=1))
        pss = ctx.enter_context(tc.tile_pool(name="pss", bufs=2, space="PSUM"))
        psx = ctx.enter_context(tc.tile_pool(name="psx", bufs=2, space="PSUM"))

        # --- persistent SBUF state ---
        wh = [wp.tile([128, D_H], BF16, tag=f"wh{i}", name=f"wh{i}") for i in range(KC)]
        wx = [wp.tile([128, D_H], BF16, tag=f"wx{k}", name=f"wx{k}") for k in range(XK)]
        for i in range(KC):
            nc.sync.dma_start(wh[i][:], wfc[D_IN + 128 * i : D_IN + 128 * (i + 1), :])
        for k in range(XK):
            nc.sync.dma_start(wx[k][:], wfc[128 * k : 128 * (k + 1), :])
        bfc_sb = wp.tile([128, JC], F32, tag="bfc", name="bfc_sb")
        nc.sync.dma_start(bfc_sb[:], bfc[:, :])

        # folded h, ping-pong parity x half: F[par][h] is [128, 4, 32]
        F = [
            [wp.tile([128, 4, BSH], BF16, tag=f"F{p}{h}", name=f"F{p}{h}") for h in range(2)]
            for p in range(2)
        ]
        nc.sync.dma_start(F[0][0][:], hf[:, 0 : 4 * BSH])
        nc.sync.dma_start(F[0][1][:], hf[:, 4 * BSH : 8 * BSH])

        # xproj result windows (double buffered) and x input windows
        XF = [wp.tile([128, WIN, JC, BSH], BF16, tag=f"XF{q}", name=f"XF{q}") for q in range(2)]
        xw = [
            [wp.tile([128, WIN * BSH], BF16, tag=f"xw{q}{k}", name=f"xw{q}{k}") for k in range(XK)]
            for q in range(2)
        ]

        # epilogue weights (DMA'd early; SBUF is plentiful)
        wo = [wp.tile([128, D_OUT], BF16, tag=f"wo{i}", name=f"wo{i}") for i in range(KC)]
        for i in range(KC):
            nc.sync.dma_start(wo[i][:], wout[128 * i : 128 * (i + 1), :])
        bout_sb = wp.tile([1, D_OUT], F32, tag="bout", name="bout_sb")
        nc.sync.dma_start(bout_sb[:], bout[:, :])
        ones_sb = wp.tile([1, BSH], F32, tag="ones", name="ones_sb")
        nc.any.memset(ones_sb[:], 1.0)
        ident = wp.tile([128, 128], BF16, tag="ident", name="ident")
        make_identity(nc, ident[:])

        def dma_xw(q, w):
            # w may be a python int or a ScalarValue expression
            for k in range(XK):
                nc.sync.dma_start(xw[q][k][:], xp[ds(w * (XK * 128) + k * 128, 128), :])

        def xproj_group(j, q_src, q_dst):
            # xproj for output chunk j of one window: [128, WIN*BSH] psum
            pt = psx.tile([128, WIN, BSH], F32, tag="px", name="px")
            for k in range(XK):
                nc.tensor.matmul(
                    pt[:],
                    wx[k][:, ts(j, 128)],
                    xw[q_src][k][:],
                    start=(k == 0),
                    stop=(k == XK - 1),
                )
            # evict with bias add + bf16 cast
            nc.scalar.activation(
                XF[q_dst][:, :, j, :], pt[:], IDENT, bias=bfc_sb[:, j : j + 1]
            )

        def scan_step(par, q_src, dt):
            # one recurrent step: F[par] -> F[par^1]
            pts = [pss.tile([128, 4, BSH], F32, tag=f"ps{h}", name=f"ps{h}") for h in range(2)]
            for h in range(2):
                for jj in range(4):
                    j = 4 * h + jj
                    for i in range(KC):
                        nc.tensor.matmul(
                            pts[h][:, jj, :],
                            wh[i][:, ts(j, 128)],
                            F[par][i // 4][:, i % 4, :],
                            start=(i == 0),
                            stop=(i == KC - 1),
                        )
            for h in range(2):
                nc.vector.tensor_add(
                    pts[h][:], pts[h][:], XF[q_src][:, dt, 4 * h : 4 * (h + 1), :]
                )
                nc.scalar.activation(F[par ^ 1][h][:], pts[h][:], TANH)

        def emit_window(q_read, q_write, q_xw):
            for dt in range(WIN):
                scan_step(dt % 2, q_read, dt)
                if dt % 2 == 0:
                    xproj_group(dt // 2, q_xw, q_write)

        # --- prologue: window 0 xproj, prefetch x windows 1 and 2 ---
        with nc.named_scope("prologue"):
            dma_xw(1, 0)
            for j in range(JC):
                xproj_group(j, 1, 0)
            dma_xw(0, 1)
            dma_xw(1, 2)

        # --- main loop: 2 windows per body ---
        if NB > 1:
            with tc.For_i(
                0,
                NB - 1,
                1,
                hint_engines=(
                    mybir.EngineType.PE,
                    mybir.EngineType.Activation,
                    mybir.EngineType.DVE,
                    mybir.EngineType.SP,
                    mybir.EngineType.Pool,
                ),
            ) as i:
                emit_window(0, 1, 0)  # scan w=2i, produce XF(2i+1)
                dma_xw(0, i * 2 + 3)
                emit_window(1, 0, 1)  # scan w=2i+1, produce XF(2i+2)
                dma_xw(1, i * 2 + 4)
        # last body peeled: no useless xproj/DMA past the end
        with nc.named_scope("last_body"):
            emit_window(0, 1, 0)
            for dt in range(WIN):
                scan_step(dt % 2, 1, dt)

        # --- epilogue ---
        with nc.named_scope("epilogue"):
            pe = ctx.enter_context(tc.tile_pool(name="pse", bufs=1, space="PSUM"))
            py = pe.tile([BSH, D_OUT], F32, tag="py", name="py")
            for i in range(KC):
                nc.tensor.matmul(
                    py[:], F[0][i // 4][:, i % 4, :], wo[i][:], start=(i == 0), stop=False
                )
            nc.tensor.matmul(py[:], ones_sb[:], bout_sb[:], start=False, stop=True)
            y_sb = wp.tile([BSH, D_OUT], F32, tag="ysb", name="y_sb")
            nc.vector.tensor_copy(y_sb[:], py[:])
            nc.sync.dma_start(y[:, :], y_sb[:])

            h_sb = wp.tile([BSH, D_H], F32, tag="hsb", name="h_sb")
            for j in range(KC):
                ptr = pe.tile([BSH, 128], BF16, tag="ptr", name="ptr")
                nc.tensor.transpose(ptr[:], F[0][j // 4][:, j % 4, :], ident[:])
                nc.vector.tensor_copy(h_sb[:, ts(j, 128)], ptr[:])
            nc.sync.dma_start(hout[:, :], h_sb[:])

    nc.compile()
    return nc


_NC_CACHE = {}


def _get_nc(T):
    if T not in _NC_CACHE:
        _NC_CACHE[T] = build_nc(T)
    return _NC_CACHE[T]


def make_in_maps(x, h, W_fc, b_fc, W_out, b_out):
    bf = ml_dtypes.bfloat16
    T = x.shape[1]
    NW = T // WIN
    NWP = NW + 3
    wfcT = np.ascontiguousarray(np.asarray(W_fc).T).astype(bf)
    woutT = np.ascontiguousarray(np.asarray(W_out).T).astype(bf)
    bfc_arr = np.ascontiguousarray(np.asarray(b_fc).reshape(JC, 128).T).astype(np.float32)
    bout_arr = np.asarray(b_out).reshape(1, D_OUT).astype(np.float32)
    x = np.asarray(x)
    h = np.asarray(h)
    in_maps = []
    for c in range(NCORES):
        xs = x[c * BSH : (c + 1) * BSH]  # [32, T, 512]
        # x_pre[w, k, p, dt*BSH+b] = xs[b, w*WIN+dt, k*128+p]
        xr = xs.reshape(BSH, NW, WIN, XK, 128).transpose(1, 3, 4, 2, 0)
        xpre = np.zeros((NWP, XK, 128, WIN * BSH), dtype=bf)
        xpre[:NW] = xr.reshape(NW, XK, 128, WIN * BSH).astype(bf)
        hs = h[c * BSH : (c + 1) * BSH]  # [32, 1024]
        hfold = (
            hs.reshape(BSH, KC, 128).transpose(2, 1, 0).reshape(128, KC * BSH).astype(bf)
        )
        in_maps.append(
            {
                "xp": np.ascontiguousarray(xpre.reshape(NWP * XK * 128, WIN * BSH)),
                "hf": np.ascontiguousarray(hfold),
                "wfc": wfcT,
                "bfc": bfc_arr,
                "wout": woutT,
                "bout": bout_arr,
            }
        )
    return in_maps


def kernel(x, h, W_fc, b_fc, W_out, b_out, trace=False):
    from concourse.bass_utils import run_bass_kernel_spmd

    T = np.asarray(x).shape[1]
    nc = _get_nc(T)
    in_maps = make_in_maps(x, h, W_fc, b_fc, W_out, b_out)
    res = run_bass_kernel_spmd(nc, in_maps, core_ids=list(range(NCORES)), trace=trace)
    yv = np.concatenate([r["y"] for r in res.results], 0)
    hv = np.concatenate([r["hout"] for r in res.results], 0)
    kernel.last_results = res
    return (yv, hv)
